# revision 7
# baseline (speedup 1.0000x reference)
"""Trainium2 Bass kernel for nn_ContConv1dDenseSim (banded continuous conv).

Math (reference):
  dt[b,l,j] = times[b,l]-times[b,j], masked to a causal band j in [l-W+1, l]
  (W = (sim_size+1)*kernel_size = 30), true_ids[b,j], and a row-validity mask.
  h = relu(dt*w1+b1)  (8 hidden), kv = (h@w2+b2) masked, reshaped (16,16)
  out[b,l,o] = sum_{j,i} features[b,j,i] * kv[b,l,j,i,o]

Factorization used here:
  G[b,j,k,o]  = sum_i f[b,j,i] * W2[k,i,o]   (k=0..7), G[b,j,8,o] = f[b,j]@B2
  A_k[j,l]    = band(l-j) * relu(dt[l,j]*w1[k]+b1[k])  (k=0..7), A_8 = band
  out[b,l,o]  = row_valid[l] * sum_{j,k} A_k[j,l] * (true_ids[j]*G[b,j,k,o])

Sharding: 8 cores = 2 batches x 4 query-row blocks of 128. Each core sees a
157-column window (128 + W-1) of keys and produces a (128,16) output block.
On-device, everything is laid out with the window column index on SBUF
partitions and the query row on the free dim, so the banded contraction
becomes 18 PSUM-accumulated matmuls (9 channels x 2 K-splits of 157).

NOTE: TRN2 PE matmuls only support a single sync-wait, so each matmul's
operands must be produced by one instruction stream: inputs are packed into
one DRAM tensor per consumer group (single DMA), and the mask channel is
staged through the vector engine so the 18 output matmuls all depend only on
the DVE queue.
"""

import numpy as np
import concourse.bass as bass
import concourse.tile as tile
import concourse.mybir as mybir
from concourse.bass_utils import run_bass_kernel_spmd
from concourse.tile_rust import add_dep_helper

F32 = mybir.dt.float32
Alu = mybir.AluOpType
Act = mybir.ActivationFunctionType

BS, L, CH, HID, KS = 2, 512, 16, 8, 5
LBLK = 128                      # query rows per core
NBLK = L // LBLK                # 4
NCORES = BS * NBLK              # 8
NKP = HID + 1                   # A channels (8 hidden + mask)
NF = NKP * CH                   # 144 G columns
NPAR = 3 + 2 * HID              # packed per-partition params columns

# test harness hooks
TRACE = False
LAST = None

_prog_cache = {}


def _build(W):
    """Build the single-core SPMD program for band width W (30 for sim=5)."""
    WIN = LBLK + W - 1          # window columns (157)
    LO = WIN - 128              # columns in the second K-split (29)
    nc = bass.Bass(trn_type="TRN2")

    # [ones; t_win] (cols 0:WIN) | [t_row; -ones] (cols WIN:WIN+128)
    dtpk = nc.declare_dram_parameter("dtpk", [2, WIN + LBLK], F32,
                                     isOutput=False)
    # feat_win^T (cols 0:WIN) | W2p with b2 column-block (cols WIN:WIN+NF)
    fw = nc.declare_dram_parameter("fw", [CH, WIN + NF], F32, isOutput=False)
    # col 0: tiw[0:128], col 1: tiw[128:WIN] (padded), col 2: row_valid,
    # cols 3:3+HID: w1 replicated, cols 3+HID:3+2*HID: b1 replicated
    par = nc.declare_dram_parameter("par", [128, NPAR], F32, isOutput=False)
    out_d = nc.declare_dram_parameter("out", [LBLK, CH], F32, isOutput=True)

    with tile.TileContext(nc) as tc:
        with (
            tc.tile_pool(name="sb", bufs=1) as sb,
            tc.tile_pool(name="ps", bufs=1, space="PSUM") as ps,
        ):
            # ---- input loads (one DMA per consumer group) ----
            t_dtpk = sb.tile([2, WIN + LBLK], F32)
            dma_a = nc.sync.dma_start(t_dtpk[:], dtpk[:])
            t_fw = sb.tile([CH, WIN + NF], F32)
            dma_b = nc.sync.dma_start(t_fw[:], fw[:])
            t_par = sb.tile([128, NPAR], F32)
            dma_c = nc.sync.dma_start(t_par[:], par[:])
            tiw_up = t_par[:, 0:1]
            tiw_lo = t_par[0:LO, 1:2]
            rv = t_par[:, 2:3]

            # TRN2 engine instructions encode at most ONE sync wait, so each
            # engine's first touch of a foreign-produced tensor must be an
            # instruction with no other new cross-engine dependency. These
            # observer ops make ACT and DVE "see" the par DMA up front; all
            # later par reads on those engines are then wait-free.
            obs_a = sb.tile([1, 1], F32)
            nc.scalar.activation(obs_a[:], t_par[0:1, 0:1], Act.Copy)
            obs_v = sb.tile([1, 1], F32)
            nc.vector.tensor_copy(obs_v[:], t_par[0:1, 0:1])

            # ---- band masks: band[jl, p] = 1 iff 0 <= jl - p <= W-1 ----
            ones_up = sb.tile([128, LBLK], F32)
            nc.vector.memset(ones_up[:], 1.0)
            bmid = sb.tile([128, LBLK], F32)
            # keep where jl - p >= 0
            nc.gpsimd.affine_select(bmid[:], ones_up[:], [[-1, LBLK]],
                                    Alu.is_ge, 0.0, base=0,
                                    channel_multiplier=1)
            band_up = sb.tile([128, LBLK], F32)
            # keep where (W-1) - jl + p >= 0
            nc.gpsimd.affine_select(band_up[:], bmid[:], [[1, LBLK]],
                                    Alu.is_ge, 0.0, base=W - 1,
                                    channel_multiplier=-1)
            ones_lo = sb.tile([LO, LBLK], F32)
            nc.vector.memset(ones_lo[:], 1.0)
            band_lo = sb.tile([LO, LBLK], F32)
            # jl = 128+q: jl-p>=0 always; keep where (W-1) - (128+q) + p >= 0
            last_gp = nc.gpsimd.affine_select(band_lo[:], ones_lo[:],
                                              [[1, LBLK]],
                                              Alu.is_ge, 0.0,
                                              base=(W - 1) - 128,
                                              channel_multiplier=-1)

            # mask channel staged through DVE: doubles as the DVE observer
            # of the gpsimd affine_selects
            a_up = sb.tile([128, NKP * LBLK], F32)
            a_lo = sb.tile([LO, NKP * LBLK], F32)
            s8 = slice(HID * LBLK, NKP * LBLK)
            nc.vector.tensor_copy(a_up[:, s8], band_up[:])
            nc.vector.tensor_copy(a_lo[:, s8], band_lo[:])

            # ---- dtT[jl, p] = t_row[p] - t_win[jl]  (rank-2 via K=2 matmul) ----
            p_dt_up = ps.tile([128, LBLK], F32)
            p_dt_lo = ps.tile([LO, LBLK], F32)
            rhs_dt = t_dtpk[:, WIN:WIN + LBLK]
            nc.tensor.matmul(p_dt_up[:], t_dtpk[:, 0:128], rhs_dt,
                             start=True, stop=True)
            nc.tensor.matmul(p_dt_lo[:], t_dtpk[:, 128:WIN], rhs_dt,
                             start=True, stop=True)

            # ---- G[jl, k*16+o] = feat_win[jl] @ W2p, then fold true_ids ----
            p_g_up = ps.tile([128, NF], F32)
            p_g_lo = ps.tile([LO, NF], F32)
            w2p_s = t_fw[:, WIN:WIN + NF]
            nc.tensor.matmul(p_g_up[:], t_fw[:, 0:128], w2p_s,
                             start=True, stop=True)
            nc.tensor.matmul(p_g_lo[:], t_fw[:, 128:WIN], w2p_s,
                             start=True, stop=True)
            g_up = sb.tile([128, NF], F32)
            nc.vector.tensor_scalar_mul(g_up[:], p_g_up[:], tiw_up)
            g_lo = sb.tile([LO, NF], F32)
            nc.vector.tensor_scalar_mul(g_lo[:], p_g_lo[:], tiw_lo)

            # ---- A channels: relu(dt*w1k + b1k) * band ----
            last_act = None
            for k in range(HID):
                s = slice(k * LBLK, (k + 1) * LBLK)
                w1s = t_par[:, 3 + k:4 + k]
                b1s = t_par[:, 3 + HID + k:4 + HID + k]
                nc.scalar.activation(a_up[:, s], p_dt_up[:], Act.Relu,
                                     bias=b1s, scale=w1s)
                nc.vector.tensor_mul(a_up[:, s], a_up[:, s], band_up[:])
                last_act = nc.scalar.activation(
                    a_lo[:, s], p_dt_lo[:], Act.Relu,
                    bias=t_par[0:LO, 3 + HID + k:4 + HID + k],
                    scale=t_par[0:LO, 3 + k:4 + k])
                nc.vector.tensor_mul(a_lo[:, s], a_lo[:, s], band_lo[:])

            # ---- out[p, o] = sum_k sum_jl A_k[jl, p] * G[jl, k*16+o] ----
            p_out = ps.tile([LBLK, CH], F32)
            last_pe = None
            for k in range(NKP):
                cs = slice(k * CH, (k + 1) * CH)
                ls = slice(k * LBLK, (k + 1) * LBLK)
                nc.tensor.matmul(p_out[:], a_up[:, ls], g_up[:, cs],
                                 start=(k == 0), stop=False)
                last_pe = nc.tensor.matmul(p_out[:], a_lo[:, ls],
                                           g_lo[:, cs],
                                           start=False, stop=(k == NKP - 1))

            # ---- row-validity fold + store ----
            o_sb = sb.tile([LBLK, CH], F32)
            last_dve = nc.vector.tensor_scalar_mul(o_sb[:], p_out[:], rv)
            dma_o = nc.sync.dma_start(out_d[:], o_sb[:])

            # The Tile kernel-tail drain waits on every outstanding
            # semaphore, but TRN2 instructions encode at most one sync
            # wait. Observe each producer from the SP sequencer with
            # single-wait nops so the drain itself needs none.
            for prod in (dma_a, dma_b, dma_c, dma_o,
                         last_gp, last_act, last_dve, last_pe):
                nop = nc.sync.nop(nofuse=True, hint="predrain_observer")
                add_dep_helper(nop.ins, prod.ins, sync=True,
                               reason="pre-drain single-wait observer")

    heavy = [(nm, type(i).__name__, len(i.sync_info.on_wait))
             for nm, i in nc.inst_map.items()
             if getattr(i, "sync_info", None) is not None
             and len(i.sync_info.on_wait) > 1
             and type(i).__name__ != "InstDrain"]
    if heavy:
        raise RuntimeError(f"multi-wait instructions would fail walrus: {heavy}")
    return nc


def kernel(times, features, lengths, true_ids, sim_size, w1, b1, w2, b2):
    global LAST
    times = np.ascontiguousarray(np.asarray(times, dtype=np.float32))
    features = np.ascontiguousarray(np.asarray(features, dtype=np.float32))
    lengths = np.asarray(lengths)
    true_ids = np.asarray(true_ids)
    sim = int(np.asarray(sim_size))
    w1 = np.asarray(w1, dtype=np.float32).reshape(-1)
    b1 = np.asarray(b1, dtype=np.float32).reshape(-1)
    w2 = np.asarray(w2, dtype=np.float32)
    b2 = np.asarray(b2, dtype=np.float32)

    W = (sim + 1) * KS
    WIN = LBLK + W - 1
    LO = WIN - 128

    if W not in _prog_cache:
        _prog_cache[W] = _build(W)
    nc = _prog_cache[W]

    w2p = np.concatenate(
        [w2.reshape(HID, CH, CH).transpose(1, 0, 2).reshape(CH, HID * CH),
         b2.reshape(CH, CH)], axis=1).astype(np.float32)

    in_maps = []
    for core in range(NCORES):
        b, blk = divmod(core, NBLK)
        l0 = blk * LBLK
        idx = np.arange(l0 - (W - 1), l0 + LBLK)
        valid = idx >= 0
        idxc = np.clip(idx, 0, L - 1)
        t_win = np.where(valid, times[b, idxc], 0.0).astype(np.float32)
        feat_win = np.where(valid[:, None], features[b, idxc, :], 0.0)
        tiw = (true_ids[b, idxc] & valid).astype(np.float32)
        t_row = times[b, l0:l0 + LBLK].astype(np.float32)
        rv = (np.arange(l0, l0 + LBLK) <=
              (sim + 1) * (int(lengths[b]) - 1)).astype(np.float32)

        dtpk = np.zeros((2, WIN + LBLK), np.float32)
        dtpk[0, :WIN] = 1.0
        dtpk[1, :WIN] = t_win
        dtpk[0, WIN:] = t_row
        dtpk[1, WIN:] = -1.0

        fw = np.zeros((CH, WIN + NF), np.float32)
        fw[:, :WIN] = feat_win.T
        fw[:, WIN:] = w2p

        par = np.zeros((128, NPAR), np.float32)
        par[:, 0] = tiw[:128]
        par[:LO, 1] = tiw[128:]
        par[:, 2] = rv
        par[:, 3:3 + HID] = w1[None, :]
        par[:, 3 + HID:3 + 2 * HID] = b1[None, :]

        in_maps.append({"dtpk": dtpk, "fw": fw, "par": par})

    res = run_bass_kernel_spmd(nc, in_maps, core_ids=list(range(NCORES)),
                               trace=TRACE)
    LAST = res

    out = np.zeros((BS, L, CH), np.float32)
    for core in range(NCORES):
        b, blk = divmod(core, NBLK)
        out[b, blk * LBLK:(blk + 1) * LBLK, :] = res.results[core]["out"]
    return out
